# revision 19
# baseline (speedup 1.0000x reference)
"""Causal self-attention with RoPE on 8 trn2 NeuronCores.

Sharding: core = (head_group g in 0..3) x (batch b in 0..1).
Each core computes qkv/RoPE/SDPA/proj for 4 heads of one batch and returns a
[T, C] partial of that batch's output (proj contracts only its 256 rows of
Wproj); the host sums the 4 head-group partials per batch and adds bproj.

Device dataflow (tuned for PE row economy + per-matmul latency):
  - all matmul inputs fp16 (x, Wqkv cast host-side); PSUM accumulates fp32
  - host passes xT = x[b].T; q^T/k^T come out as [d, t] tiles
  - Wq/Wk columns permuted head-contiguous per 128-row j-tile:
    [h_e(32) h_o(32) | h'_e(32) h'_o(32)], so RoPE is 2 full-tile mults
    (cos/sin tables replicated per 32-row block) + 4 strided adds that land
    the rotated values DIRECTLY in the score-ready layout - no fixup copies
  - ONE shared fp16 cos/sin table pair for q and k; the 1/sqrt(D) score scale
    is folded into the exp activation's free scale parameter (x*0.125)
  - qT is stored BLOCK-DIAGONAL [128, 4 slots, t]: slot 2jt holds head 2jt in
    rows 0:64 (rows 64:128 zero), slot 2jt+1 holds head 2jt+1 in rows 64:128.
    Scores for a head pair are then ONE [128]-contraction matmul with
    free=2*512 at full PE rate (vs 2 half-rate K=64 matmuls)
  - causal: diagonal k-tile j restricts score/exp/av APs to q >= j*128
    (0.53x dense, the per-q-tile ideal) and only the [128,128] triangle
    block gets a mask multiply (DVE, fp16 2x)
  - V stored [k, 4*(64 data | 64 ones)]: attn@V_aug gives y and the softmax
    denominator in one accumulating matmul; normalization is a DVE
    reciprocal (no ACT table swaps - ACT does exp only, all run long)
  - engine load balance: PE matmuls; ACT exp only; DVE RoPE/tri-mask/
    normalize; Pool(gpsimd) v-copy, psum->fp16 out casts, memsets; Sync
    carries in/out DMA triggers with inputs split across idle engine queues
    at startup (spreads descriptor-gen serialization)
  - PSUM: pav accumulators own tag A (2 slots); ALL transient psum tiles
    (phase1 q/k, psv, scores, proj) share the tag-S ring so no transient
    alloc ever blocks on a live accumulator
  - emission is software-pipelined (phase1/SDPA-core/normalize/proj
    interleaved) so PE work covers the exp chain and RoPE tails

No numerics tricks beyond fp16 inputs: exp without max-subtraction (scores
~N(0,1) after scale, far from fp32 overflow).
"""

import os
import sys

import numpy as np

for _p in ("/opt/trn_rl_repo", "/root/.axon_site/_ro/trn_rl_repo"):
    if os.path.isdir(_p) and _p not in sys.path:
        sys.path.append(_p)

import concourse.bass as bass  # noqa: E402
import concourse.mybir as mybir  # noqa: E402
import concourse.tile as tile  # noqa: E402
from concourse import bacc  # noqa: E402
from concourse.bass_utils import run_bass_kernel_spmd  # noqa: E402

B = 2
T = 2048
C = 1024
H = 16
D = 64
ROPE_BASE = 10000.0

HG = 4            # heads per core
J = HG * D        # 256 local qkv columns per tensor
NCORES = 8
RC = 512          # row chunk (phase 1 free dim / q chunk)
KT = 128          # k tile
F32 = mybir.dt.float32
FP16 = mybir.dt.float16

_nc_cache = None


def _bcast2(ap_2d, n):
    """[128, F] slice -> [128, n(bcast), F] via a zero-stride middle dim."""
    return bass.AP(
        tensor=ap_2d.tensor, offset=ap_2d.offset,
        ap=[ap_2d.ap[0], [0, n], ap_2d.ap[-1]])


def _build(debug=False):
    nc = bacc.Bacc(None, target_bir_lowering=False)

    xt = nc.dram_tensor("xt", [C, T], FP16, kind="ExternalInput")
    wq = nc.dram_tensor("wq", [C, J], FP16, kind="ExternalInput")
    wk = nc.dram_tensor("wk", [C, J], FP16, kind="ExternalInput")
    wv = nc.dram_tensor("wv", [C, J], FP16, kind="ExternalInput")
    wp = nc.dram_tensor("wp", [J, C], FP16, kind="ExternalInput")
    # trig tables: cos/sin rows replicated per 32-block, shared by q and k
    t1 = nc.dram_tensor("t1", [128, T], FP16, kind="ExternalInput")
    t2 = nc.dram_tensor("t2", [128, T], FP16, kind="ExternalInput")
    tri = nc.dram_tensor("tri", [128, KT], FP16, kind="ExternalInput")
    out = nc.dram_tensor("out", [T, C], FP16, kind="ExternalOutput")
    if debug:
        dq = nc.dram_tensor("dq", [128, 4 * RC], FP16, kind="ExternalOutput")
        dk = nc.dram_tensor("dk", [128, 2 * RC], FP16, kind="ExternalOutput")
        dv = nc.dram_tensor("dv", [128, HG * 128], FP16, kind="ExternalOutput")
        dy0 = nc.dram_tensor("dy0", [128, RC], FP16, kind="ExternalOutput")
        dy3 = nc.dram_tensor("dy3", [128, RC], FP16, kind="ExternalOutput")
        drec = nc.dram_tensor("drec", [64, 2 * RC], F32, kind="ExternalOutput")

    n_rc = T // RC            # 4
    n_ct = C // 128           # 8 contraction tiles
    n_vt = T // KT            # 16 v tiles

    with tile.TileContext(nc) as tc:
        with (
            tc.tile_pool(name="persist", bufs=1) as persist,
            tc.tile_pool(name="xc", bufs=3) as xcp,
            tc.tile_pool(name="tmp", bufs=4) as tmpp,
            tc.tile_pool(name="expp", bufs=10) as expp,
            tc.tile_pool(name="npool", bufs=3) as npool,
            tc.tile_pool(name="ps", bufs=2, space="PSUM") as psp,
        ):
            # ---- persistent tiles ----
            wq_sb = persist.tile([128, n_ct, J], FP16, tag="wq")
            wk_sb = persist.tile([128, n_ct, J], FP16, tag="wk")
            wv_sb = persist.tile([128, n_ct, J], FP16, tag="wv")
            t1_sb = persist.tile([128, T], FP16, tag="t1")
            t2_sb = persist.tile([128, T], FP16, tag="t2")
            tri_sb = persist.tile([128, KT], FP16, tag="tri")
            wp_sb = persist.tile([128, 2, C], FP16, tag="wp")

            # qALL[rc]: [128, 4 slots, RC] block-diagonal (see module doc)
            qALL = [persist.tile([128, 4, RC], FP16, tag=f"qA{r}", name=f"qA{r}")
                    for r in range(n_rc)]
            kALL = [persist.tile([128, 2, RC], FP16, tag=f"kA{r}", name=f"kA{r}")
                    for r in range(n_rc)]
            yT = [[persist.tile([128, RC], FP16, tag=f"yT{j}_{r}", name=f"yT{j}_{r}")
                   for r in range(n_rc)] for j in range(2)]
            # v tiles: [128, HG*128] fp16; head l data at cols l*128..+64, ones after
            v_sb = [persist.tile([128, HG * 128], FP16, tag=f"v{i}", name=f"v{i}")
                    for i in range(n_vt)]
            xall = [xcp.tile([128, n_ct, RC], FP16, tag="xc", name=f"x{r}")
                    for r in range(n_rc)]

            xtr = xt.rearrange("(co p) t -> p co t", p=128)

            # ---- input DMAs spread across the 3 DMA-capable engine queues
            # (sync/SP, scalar/ACT, gpsimd/Pool), strictly need-ordered:
            # startup is BANDWIDTH-bound (~350GB/s across all queues), so the
            # first wave holds only what the first accumulations need, with
            # xall[0] split by c-tile so the q accumulation starts as tiles
            # arrive.
            nc.sync.dma_start(wq_sb, wq.rearrange("(co p) j -> p co j", p=128))
            nc.gpsimd.dma_start(xall[0][:, 0:4, :], xtr[:, 0:4, 0:RC])
            nc.sync.dma_start(xall[0][:, 4:8, :], xtr[:, 4:8, 0:RC])
            nc.scalar.dma_start(tri_sb, tri[:, :])
            nc.scalar.dma_start(wk_sb, wk.rearrange("(co p) j -> p co j", p=128))
            nc.gpsimd.dma_start(t1_sb, t1[:, :])
            nc.gpsimd.dma_start(t2_sb, t2[:, :])
            nc.sync.dma_start(wv_sb, wv.rearrange("(co p) j -> p co j", p=128))
            nc.gpsimd.dma_start(xall[1][:, 0:4, :], xtr[:, 0:4, RC:2 * RC])
            nc.sync.dma_start(xall[1][:, 4:8, :], xtr[:, 4:8, RC:2 * RC])
            nc.scalar.dma_start(wp_sb, wp.rearrange("(jt p) n -> p jt n", p=128))
            # xall[2], xall[3] prefetched inside the pipeline below (on sync)

            # zero the off-diagonal q half-blocks (disjoint from the RoPE
            # write region, so RoPE never waits on these) and set the V ones
            # columns (before phase 2). On Pool, ordered by first-use time.
            def ones_for(lo, hi):
                for i in range(lo, hi):
                    oap = v_sb[i].rearrange("p (l x) -> p l x", x=128)[:, :, D:128]
                    nc.gpsimd.memset(oap, 1.0)

            def qzero(r):
                qv = qALL[r].rearrange("p (a b) t -> p a b t", b=2)
                nc.gpsimd.memset(qv[64:128, :, 0, :], 0.0)
                nc.gpsimd.memset(qv[0:64, :, 1, :], 0.0)

            ones_for(0, 4)
            for r in range(n_rc):
                qzero(r)
            ones_for(4, n_vt)

            # ---------------- emission helpers ----------------
            def emit_phase1(rc):
                """qkv + RoPE for row chunk rc."""
                rcs = slice(rc * RC, (rc + 1) * RC)
                if rc + 2 < n_rc:
                    nc.sync.dma_start(
                        xall[rc + 2], xtr[:, :, (rc + 2) * RC:(rc + 3) * RC])

                for (w_sb, kind) in ((wq_sb, "q"), (wk_sb, "k")):
                    ps = psp.tile([128, 2, RC], F32, tag="S", name=f"p1{kind}_{rc}")
                    for jt in range(2):
                        for c in range(n_ct):
                            nc.tensor.matmul(
                                ps[:, jt, :],
                                w_sb[:, c, jt * 128:(jt + 1) * 128],
                                xall[rc][:, c, :],
                                start=(c == 0), stop=(c == n_ct - 1))
                    # RoPE on psum rows [h0e h1e | h0o h1o]:
                    #   A  = ps * cos           (natural rows)
                    #   B~ = swap64(ps) * sin   (2 half mults; PSUM-source
                    #                            partition shift is free)
                    # then 4 half adds with base-aligned SBUF inputs; the
                    # OUTPUT base is free, so results land head-contiguous
                    # ([he(32) ho(32)] per head) with no fixup copies.
                    T1s = _bcast2(t1_sb[:, rcs], 2)
                    T2s = _bcast2(t2_sb[:, rcs], 2)
                    T2lo = bass.AP(tensor=T2s.tensor, offset=T2s.offset,
                                   ap=[[T2s.ap[0][0], 64], [0, 2], [1, RC]])
                    A = tmpp.tile([128, 2, RC], FP16, tag="A", name=f"A{kind}{rc}")
                    Bt = tmpp.tile([128, 2, RC], FP16, tag="B", name=f"B{kind}{rc}")
                    nc.vector.tensor_tensor(A, ps, T1s, mybir.AluOpType.mult)
                    nc.vector.tensor_tensor(
                        Bt[0:64], ps[64:128], T2lo, mybir.AluOpType.mult)
                    nc.vector.tensor_tensor(
                        Bt[64:128], ps[0:64], T2lo, mybir.AluOpType.mult)
                    if kind == "q":
                        dsv = qALL[rc].rearrange("p (a b) t -> p a b t", b=2)
                        d_h0e = dsv[0:32, :, 0, :]
                        d_h0o = dsv[32:64, :, 0, :]
                        d_h1e = dsv[64:96, :, 1, :]
                        d_h1o = dsv[96:128, :, 1, :]
                    else:
                        d_h0e = kALL[rc][0:32, :, :]
                        d_h0o = kALL[rc][32:64, :, :]
                        d_h1e = kALL[rc][64:96, :, :]
                        d_h1o = kALL[rc][96:128, :, :]
                    # rows of A/B~: 0:32=h0e, 32:64=h1e, 64:96=h0o, 96:128=h1o
                    nc.vector.tensor_tensor(
                        d_h0e, A[0:32], Bt[0:32], mybir.AluOpType.subtract)
                    nc.vector.tensor_tensor(
                        d_h1e, A[32:64], Bt[32:64], mybir.AluOpType.subtract)
                    nc.vector.tensor_tensor(
                        d_h0o, A[64:96], Bt[64:96], mybir.AluOpType.add)
                    nc.vector.tensor_tensor(
                        d_h1o, A[96:128], Bt[96:128], mybir.AluOpType.add)

                # v for this row chunk: 4 sub r-tiles in one 2-bank psum
                psv = psp.tile([128, 4, J], F32, tag="S", name=f"pv_{rc}")
                for sub in range(RC // KT):
                    for c in range(n_ct):
                        nc.tensor.matmul(
                            psv[:, sub, :],
                            xall[rc][:, c, sub * KT:(sub + 1) * KT],
                            wv_sb[:, c, :],
                            start=(c == 0), stop=(c == n_ct - 1))
                for sub in range(RC // KT):
                    vt = v_sb[rc * (RC // KT) + sub]
                    nc.vector.tensor_copy(
                        vt.rearrange("p (l x) -> p l x", x=128)[:, :, 0:D],
                        psv[:, sub, :].rearrange("p (l d) -> p l d", l=HG))

            pavs = {}

            def emit_core(qc):
                """SDPA kt-loop for q-chunk qc, both head pairs concurrently."""
                nk = 4 * qc + 4
                qvs = [qALL[qc].rearrange("p (a b) t -> p a b t", b=2)[:, jt, :, :]
                       for jt in range(2)]
                pav = [psp.tile([128, 2, RC], F32, tag="A", name=f"av{jt}_{qc}")
                       for jt in range(2)]
                for kt in range(nk):
                    j = kt - 4 * qc
                    qoff = max(0, j) * KT
                    for jt in range(2):
                        ps_s = psp.tile([128, 2, RC], F32, tag="S",
                                        name=f"s{jt}_{qc}_{kt}")
                        kap = kALL[kt // 4][:, jt, (kt % 4) * KT:(kt % 4 + 1) * KT]
                        for lh in range(2):
                            nc.tensor.matmul(
                                ps_s[:, lh, qoff:RC], kap,
                                qvs[jt][:, lh, qoff:RC],
                                start=True, stop=True)
                        e = expp.tile([128, 2, RC], FP16, tag="e",
                                      name=f"e{jt}_{qc}_{kt}")
                        nc.scalar.activation(
                            e[:, :, qoff:RC], ps_s[:, :, qoff:RC],
                            mybir.ActivationFunctionType.Exp, scale=0.125)
                        if j >= 0:  # diagonal tile: mask the triangle block
                            # on Pool: SBUF-only op, and Pool is idle
                            # mid-run, so the exp->AV chain never queues
                            # behind bulky DVE work
                            tslice = e[:, :, qoff:qoff + KT]
                            nc.gpsimd.tensor_tensor(
                                tslice, tslice, _bcast2(tri_sb[:, :], 2),
                                mybir.AluOpType.mult)
                        for lh in range(2):
                            hcol = (2 * jt + lh) * 128
                            nc.tensor.matmul(
                                pav[jt][:, lh, qoff:RC],
                                v_sb[kt][:, hcol:hcol + 128],
                                e[:, lh, qoff:RC],
                                start=(kt == 0), stop=(kt == nk - 1))
                pavs[qc] = pav

            def emit_norm(qc):
                """normalize both head pairs. Stage pav's y rows and
                denominator rows to base-0 SBUF tiles (jt0 via ACT, jt1 via
                DVE, in parallel) so the psum accumulators release after the
                copies. reciprocal_approx_fast REQUIRES a base-partition-0
                fp32 SBUF input (partition-offset APs silently misread on
                HW). The scale mults run on Pool (SBUF-only), off DVE."""
                pav = pavs.pop(qc)
                yus, dens = [], []
                for jt in range(2):
                    yu = npool.tile([64, 2, RC], FP16, tag=f"yu{jt}",
                                    name=f"yu{jt}_{qc}")
                    den = npool.tile([64, 2, RC], F32, tag=f"den{jt}",
                                     name=f"dn{jt}_{qc}")
                    if jt == 0:
                        nc.scalar.copy(yu, pav[jt][0:64, :, :])
                        nc.scalar.copy(den, pav[jt][64:128, :, :])
                    else:
                        nc.vector.tensor_copy(yu, pav[jt][0:64, :, :])
                        nc.vector.tensor_copy(den, pav[jt][64:128, :, :])
                    yus.append(yu)
                    dens.append(den)
                for jt in range(2):
                    rec = npool.tile([64, 2, RC], F32, tag=f"rec{jt}",
                                     name=f"r{jt}_{qc}")
                    nc.vector.reciprocal_approx_fast(out=rec, in_=dens[jt])
                    if debug and qc == 0 and jt == 0:
                        nc.sync.dma_start(drec[:, :], rec[:, :, :])
                    nc.gpsimd.tensor_tensor(
                        yT[jt][qc][0:64, :], yus[jt][:, 0, :],
                        rec[:, 0, :], mybir.AluOpType.mult)
                    nc.gpsimd.tensor_tensor(
                        yT[jt][qc][64:128, :], yus[jt][:, 1, :],
                        rec[:, 1, :], mybir.AluOpType.mult)

            def emit_proj(qc):
                """output projection partial for q-chunk qc + store."""
                for rt in range(4 * qc, 4 * qc + 4):
                    rs = slice(rt * 128, (rt + 1) * 128)
                    ro = (rt % 4) * 128
                    po = psp.tile([128, 2 * RC], F32, tag="S", name=f"po_{rt}")
                    for nt in range(2):
                        ns = slice(nt * 512, (nt + 1) * 512)
                        nc.tensor.matmul(po[:, ns], yT[0][qc][:, ro:ro + 128],
                                         wp_sb[:, 0, ns], start=True, stop=False)
                        nc.tensor.matmul(po[:, ns], yT[1][qc][:, ro:ro + 128],
                                         wp_sb[:, 1, ns], start=False, stop=True)
                    # PSUM->SBUF fp16 cast on DVE (idle during core phases
                    # now that norm mults moved to Pool)
                    o_sb = npool.tile([128, 2 * RC], FP16, tag="o_sb")
                    nc.vector.tensor_copy(o_sb, po)
                    nc.gpsimd.dma_start(out[rs, :], o_sb)

            # ---------------- interleaved schedule ----------------
            # norm(qc) is emitted RIGHT AFTER core(qc): its DVE ops then sit
            # ahead of the next phase1's bulky RoPE work in the DVE FIFO, so
            # the pav accumulators release quickly for core(qc+1). PE covers
            # the norm chain with phase1/proj matmuls. proj(2) goes before
            # norm(3) so the PE stays warm while the last norm chain runs.
            emit_phase1(0)
            emit_phase1(1)
            emit_core(0)
            emit_norm(0)
            emit_phase1(2)
            emit_core(1)
            emit_norm(1)
            emit_phase1(3)
            emit_proj(0)
            emit_core(2)
            emit_norm(2)
            emit_proj(1)
            emit_core(3)
            emit_proj(2)
            emit_norm(3)
            emit_proj(3)

            if debug:
                nc.sync.dma_start(dq[:, :], qALL[0].rearrange("p a t -> p (a t)"))
                nc.sync.dma_start(dk[:, :], kALL[0].rearrange("p a t -> p (a t)"))
                nc.sync.dma_start(dv[:, :], v_sb[0])
                nc.sync.dma_start(dy0[:, :], yT[0][0])
                nc.sync.dma_start(dy3[:, :], yT[0][3])

    nc.finalize()
    return nc


def _host_inputs(x, Wqkv, Wproj):
    x = np.asarray(x, dtype=np.float32)
    Wqkv = np.asarray(Wqkv, dtype=np.float32)
    Wproj = np.asarray(Wproj, dtype=np.float32)

    # RoPE tables (match reference: theta_i = base^(-2i/D), freqs = outer(t, theta))
    dim_idx = np.arange(D // 2, dtype=np.float32)
    theta = 1.0 / (ROPE_BASE ** (2.0 * dim_idx / D))
    t = np.arange(T, dtype=np.float32)
    freqs = np.outer(t, theta).astype(np.float32)         # [T, 32]
    cos32 = np.cos(freqs).T.astype(np.float32)            # [32, T]
    sin32 = np.sin(freqs).T.astype(np.float32)
    t1_h = np.ascontiguousarray(np.tile(cos32, (4, 1)).astype(np.float16))
    t2_h = np.ascontiguousarray(np.tile(sin32, (4, 1)).astype(np.float16))

    # causal triangle mask for the diagonal 128x128 block: keep k <= q
    kk = np.arange(KT)[:, None]
    qq = np.arange(KT)[None, :]
    tri_h = np.ascontiguousarray((kk <= qq).astype(np.float16))

    # q/k column permutation: j-tile jt holds heads (2jt, 2jt+1) as
    # [h_e(32) h'_e(32) | h_o(32) h'_o(32)] (evens top half, odds bottom)
    def qk_perm(g):
        idx = np.empty(J, dtype=np.int64)
        for jt in range(2):
            for p in range(128):
                if p < 32:
                    lh, dd = 2 * jt, 2 * p
                elif p < 64:
                    lh, dd = 2 * jt + 1, 2 * (p - 32)
                elif p < 96:
                    lh, dd = 2 * jt, 2 * (p - 64) + 1
                else:
                    lh, dd = 2 * jt + 1, 2 * (p - 96) + 1
                idx[jt * 128 + p] = (4 * g + lh) * D + dd
        return idx

    xT = [np.ascontiguousarray(x[b].T.astype(np.float16)) for b in range(B)]
    in_maps = []
    for core in range(NCORES):
        g, b = core // 2, core % 2
        perm = qk_perm(g)
        wq_g = np.ascontiguousarray(Wqkv[:, perm].astype(np.float16))
        wk_g = np.ascontiguousarray(Wqkv[:, C + perm].astype(np.float16))
        vcols = np.arange(4 * g * D, 4 * g * D + J)
        wv_g = np.ascontiguousarray(Wqkv[:, 2 * C + vcols].astype(np.float16))
        wp_g = np.ascontiguousarray(
            Wproj[4 * g * D: 4 * g * D + J, :].astype(np.float16))
        in_maps.append({
            "xt": xT[b], "wq": wq_g, "wk": wk_g, "wv": wv_g, "wp": wp_g,
            "t1": t1_h, "t2": t2_h, "tri": tri_h,
        })
    return in_maps


def kernel(x, Wqkv, bqkv, Wproj, bproj, _want_results=False):
    global _nc_cache
    if _nc_cache is None:
        _nc_cache = _build()
    in_maps = _host_inputs(x, Wqkv, Wproj)
    res = run_bass_kernel_spmd(_nc_cache, in_maps, list(range(NCORES)))

    bqkv = np.asarray(bqkv, dtype=np.float32)
    bproj = np.asarray(bproj, dtype=np.float32)
    out = np.zeros((B, T, C), dtype=np.float32)
    for core in range(NCORES):
        g, b = core // 2, core % 2
        out[b] += res.results[core]["out"]
    out += bproj[None, None, :]
    if _want_results:
        return out, res
    return out


# revision 28
# speedup vs baseline: 1.0441x; 1.0441x over previous
"""Causal self-attention with RoPE on 8 trn2 NeuronCores.

Sharding: core = (head_group g in 0..3) x (batch b in 0..1).
Each core computes qkv/RoPE/SDPA/proj for 4 heads of one batch and returns a
[T, C] partial of that batch's output (proj contracts only its 256 rows of
Wproj); the host sums the 4 head-group partials per batch and adds bproj.

Device dataflow (tuned for PE row economy + per-matmul latency):
  - all matmul inputs fp16 (x, Wqkv cast host-side); PSUM accumulates fp32
  - host passes xT = x[b].T; q^T/k^T come out as [d, t] tiles
  - Wq/Wk columns permuted head-contiguous per 128-row j-tile:
    [h_e(32) h_o(32) | h'_e(32) h'_o(32)], so RoPE is 2 full-tile mults
    (cos/sin tables replicated per 32-row block) + 4 strided adds that land
    the rotated values DIRECTLY in the score-ready layout - no fixup copies
  - ONE shared fp16 cos/sin table pair for q and k; the 1/sqrt(D) score scale
    is folded into the exp activation's free scale parameter (x*0.125)
  - qT is stored BLOCK-DIAGONAL [128, 4 slots, t]: slot 2jt holds head 2jt in
    rows 0:64 (rows 64:128 zero), slot 2jt+1 holds head 2jt+1 in rows 64:128.
    Scores for a head pair are then ONE [128]-contraction matmul with
    free=2*512 at full PE rate (vs 2 half-rate K=64 matmuls)
  - causal: diagonal k-tile j restricts score/exp/av APs to q >= j*128
    (0.53x dense, the per-q-tile ideal) and only the [128,128] triangle
    block gets a mask multiply (DVE, fp16 2x)
  - V stored [k, 4*(64 data | 64 ones)]: attn@V_aug gives y and the softmax
    denominator in one accumulating matmul; normalization is a DVE
    reciprocal (no ACT table swaps - ACT does exp only, all run long)
  - engine load balance: PE matmuls; ACT exp only; DVE RoPE/tri-mask/
    normalize; Pool(gpsimd) v-copy, psum->fp16 out casts, memsets; Sync
    carries in/out DMA triggers with inputs split across idle engine queues
    at startup (spreads descriptor-gen serialization)
  - PSUM: pav accumulators own tag A (2 slots); ALL transient psum tiles
    (phase1 q/k, psv, scores, proj) share the tag-S ring so no transient
    alloc ever blocks on a live accumulator
  - emission is software-pipelined (phase1/SDPA-core/normalize/proj
    interleaved) so PE work covers the exp chain and RoPE tails

No numerics tricks beyond fp16 inputs: exp without max-subtraction (scores
~N(0,1) after scale, far from fp32 overflow).
"""

import os
import sys

import numpy as np

for _p in ("/opt/trn_rl_repo", "/root/.axon_site/_ro/trn_rl_repo"):
    if os.path.isdir(_p) and _p not in sys.path:
        sys.path.append(_p)

import concourse.bass as bass  # noqa: E402
import concourse.mybir as mybir  # noqa: E402
import concourse.tile as tile  # noqa: E402
from concourse import bacc  # noqa: E402
from concourse.bass_utils import run_bass_kernel_spmd  # noqa: E402

B = 2
T = 2048
C = 1024
H = 16
D = 64
ROPE_BASE = 10000.0

HG = 4            # heads per core
J = HG * D        # 256 local qkv columns per tensor
NCORES = 8
RC = 512          # row chunk (phase 1 free dim / q chunk)
KT = 128          # k tile
F32 = mybir.dt.float32
FP16 = mybir.dt.float16

_nc_cache = None


def _bcast2(ap_2d, n):
    """[128, F] slice -> [128, n(bcast), F] via a zero-stride middle dim."""
    return bass.AP(
        tensor=ap_2d.tensor, offset=ap_2d.offset,
        ap=[ap_2d.ap[0], [0, n], ap_2d.ap[-1]])


def _build(debug=False):
    nc = bacc.Bacc(None, target_bir_lowering=False)

    xt = nc.dram_tensor("xt", [C, T], FP16, kind="ExternalInput")
    wq = nc.dram_tensor("wq", [C, J], FP16, kind="ExternalInput")
    wk = nc.dram_tensor("wk", [C, J], FP16, kind="ExternalInput")
    wv = nc.dram_tensor("wv", [C, J], FP16, kind="ExternalInput")
    wp = nc.dram_tensor("wp", [J, C], FP16, kind="ExternalInput")
    # trig tables: cos/sin rows replicated per 32-block, shared by q and k
    t1 = nc.dram_tensor("t1", [128, T], FP16, kind="ExternalInput")
    t2 = nc.dram_tensor("t2", [128, T], FP16, kind="ExternalInput")
    # causal penalty for the diagonal 128x128 score block, applied as an
    # extra accumulating matmul ident.T @ mpen (mpen = -200 where k > q):
    # exp then underflows masked entries to zero - no post-exp mask op at all
    ident = nc.dram_tensor("ident", [128, KT], FP16, kind="ExternalInput")
    mpen = nc.dram_tensor("mpen", [128, KT], FP16, kind="ExternalInput")
    out = nc.dram_tensor("out", [T, C], FP16, kind="ExternalOutput")
    if debug:
        dq = nc.dram_tensor("dq", [128, 4 * RC], FP16, kind="ExternalOutput")
        dk = nc.dram_tensor("dk", [128, 2 * RC], FP16, kind="ExternalOutput")
        dv = nc.dram_tensor("dv", [128, HG * 128], FP16, kind="ExternalOutput")
        dy0 = nc.dram_tensor("dy0", [128, RC], FP16, kind="ExternalOutput")
        dy3 = nc.dram_tensor("dy3", [128, RC], FP16, kind="ExternalOutput")
        drec = nc.dram_tensor("drec", [64, 2 * RC], F32, kind="ExternalOutput")

    n_rc = T // RC            # 4
    n_ct = C // 128           # 8 contraction tiles
    n_vt = T // KT            # 16 v tiles

    with tile.TileContext(nc) as tc:
        with (
            tc.tile_pool(name="persist", bufs=1) as persist,
            tc.tile_pool(name="xc", bufs=3) as xcp,
            tc.tile_pool(name="tmp", bufs=4) as tmpp,
            tc.tile_pool(name="expp", bufs=10) as expp,
            tc.tile_pool(name="npool", bufs=3) as npool,
            tc.tile_pool(name="ps", bufs=2, space="PSUM") as psp,
        ):
            # ---- persistent tiles ----
            wq_sb = persist.tile([128, n_ct, J], FP16, tag="wq")
            wk_sb = persist.tile([128, n_ct, J], FP16, tag="wk")
            wv_sb = persist.tile([128, n_ct, J], FP16, tag="wv")
            t1_sb = persist.tile([128, T], FP16, tag="t1")
            t2_sb = persist.tile([128, T], FP16, tag="t2")
            ident_sb = persist.tile([128, KT], FP16, tag="ident")
            mpen_sb = persist.tile([128, KT], FP16, tag="mpen")
            wp_sb = persist.tile([128, 2, C], FP16, tag="wp")

            # qALL[rc]: [128, 4 slots, RC] block-diagonal (see module doc)
            qALL = [persist.tile([128, 4, RC], FP16, tag=f"qA{r}", name=f"qA{r}")
                    for r in range(n_rc)]
            kALL = [persist.tile([128, 2, RC], FP16, tag=f"kA{r}", name=f"kA{r}")
                    for r in range(n_rc)]
            yT = [[persist.tile([128, RC], FP16, tag=f"yT{j}_{r}", name=f"yT{j}_{r}")
                   for r in range(n_rc)] for j in range(2)]
            # v tiles: [128, HG*128] fp16; head l data at cols l*128..+64, ones after
            v_sb = [persist.tile([128, HG * 128], FP16, tag=f"v{i}", name=f"v{i}")
                    for i in range(n_vt)]
            xall = [xcp.tile([128, n_ct, RC], FP16, tag="xc", name=f"x{r}")
                    for r in range(n_rc)]

            xtr = xt.rearrange("(co p) t -> p co t", p=128)

            # ---- input DMAs spread across the 3 DMA-capable engine queues
            # (sync/SP, scalar/ACT, gpsimd/Pool), strictly need-ordered AND
            # byte-balanced: each queue sustains only ~150-220GB/s, so the
            # ~7MB input must be split evenly; xall[0] is split by c-tile so
            # the first q accumulation starts as tiles arrive.
            nc.sync.dma_start(wq_sb, wq.rearrange("(co p) j -> p co j", p=128))
            nc.gpsimd.dma_start(xall[0][:, 0:4, :], xtr[:, 0:4, 0:RC])
            nc.sync.dma_start(xall[0][:, 4:8, :], xtr[:, 4:8, 0:RC])
            nc.scalar.dma_start(ident_sb, ident[:, :])
            nc.scalar.dma_start(mpen_sb, mpen[:, :])
            nc.scalar.dma_start(wk_sb, wk.rearrange("(co p) j -> p co j", p=128))
            nc.gpsimd.dma_start(t1_sb, t1[:, :])
            nc.scalar.dma_start(t2_sb, t2[:, :])
            nc.scalar.dma_start(wv_sb, wv.rearrange("(co p) j -> p co j", p=128))
            nc.gpsimd.dma_start(xall[1][:, 0:4, :], xtr[:, 0:4, RC:2 * RC])
            nc.sync.dma_start(xall[1][:, 4:8, :], xtr[:, 4:8, RC:2 * RC])
            nc.scalar.dma_start(wp_sb, wp.rearrange("(jt p) n -> p jt n", p=128))
            # xall[2], xall[3] prefetched inside the pipeline below (on sync)

            # zero the off-diagonal q half-blocks (disjoint from the RoPE
            # write region, so RoPE never waits on these) and set the V ones
            # columns (before phase 2). On Pool, ordered by first-use time.
            def ones_for(lo, hi):
                for i in range(lo, hi):
                    oap = v_sb[i].rearrange("p (l x) -> p l x", x=128)[:, :, D:128]
                    nc.gpsimd.memset(oap, 1.0)

            def qzero(r):
                qv = qALL[r].rearrange("p (a b) t -> p a b t", b=2)
                nc.gpsimd.memset(qv[64:128, :, 0, :], 0.0)
                nc.gpsimd.memset(qv[0:64, :, 1, :], 0.0)

            ones_for(0, 4)
            for r in range(n_rc):
                qzero(r)
            ones_for(4, n_vt)

            # ---------------- emission helpers ----------------
            def emit_phase1(rc):
                """qkv + RoPE for row chunk rc."""
                rcs = slice(rc * RC, (rc + 1) * RC)
                if rc + 2 < n_rc:
                    nc.sync.dma_start(
                        xall[rc + 2], xtr[:, :, (rc + 2) * RC:(rc + 3) * RC])

                for (w_sb, kind) in ((wq_sb, "q"), (wk_sb, "k")):
                    ps = psp.tile([128, 2, RC], F32, tag="S", name=f"p1{kind}_{rc}")
                    for jt in range(2):
                        for c in range(n_ct):
                            nc.tensor.matmul(
                                ps[:, jt, :],
                                w_sb[:, c, jt * 128:(jt + 1) * 128],
                                xall[rc][:, c, :],
                                start=(c == 0), stop=(c == n_ct - 1))
                    # RoPE on psum rows [h0e h1e | h0o h1o]:
                    #   A  = ps * cos           (natural rows)
                    #   B~ = swap64(ps) * sin   (2 half mults; PSUM-source
                    #                            partition shift is free)
                    # then 4 half adds with base-aligned SBUF inputs; the
                    # OUTPUT base is free, so results land head-contiguous
                    # ([he(32) ho(32)] per head) with no fixup copies.
                    T1s = _bcast2(t1_sb[:, rcs], 2)
                    T2s = _bcast2(t2_sb[:, rcs], 2)
                    T2lo = bass.AP(tensor=T2s.tensor, offset=T2s.offset,
                                   ap=[[T2s.ap[0][0], 64], [0, 2], [1, RC]])
                    A = tmpp.tile([128, 2, RC], FP16, tag="A", name=f"A{kind}{rc}")
                    Bt = tmpp.tile([128, 2, RC], FP16, tag="B", name=f"B{kind}{rc}")
                    nc.vector.tensor_tensor(A, ps, T1s, mybir.AluOpType.mult)
                    nc.vector.tensor_tensor(
                        Bt[0:64], ps[64:128], T2lo, mybir.AluOpType.mult)
                    nc.vector.tensor_tensor(
                        Bt[64:128], ps[0:64], T2lo, mybir.AluOpType.mult)
                    if kind == "q":
                        dsv = qALL[rc].rearrange("p (a b) t -> p a b t", b=2)
                        d_h0e = dsv[0:32, :, 0, :]
                        d_h0o = dsv[32:64, :, 0, :]
                        d_h1e = dsv[64:96, :, 1, :]
                        d_h1o = dsv[96:128, :, 1, :]
                    else:
                        d_h0e = kALL[rc][0:32, :, :]
                        d_h0o = kALL[rc][32:64, :, :]
                        d_h1e = kALL[rc][64:96, :, :]
                        d_h1o = kALL[rc][96:128, :, :]
                    # rows of A/B~: 0:32=h0e, 32:64=h1e, 64:96=h0o, 96:128=h1o
                    nc.vector.tensor_tensor(
                        d_h0e, A[0:32], Bt[0:32], mybir.AluOpType.subtract)
                    nc.vector.tensor_tensor(
                        d_h1e, A[32:64], Bt[32:64], mybir.AluOpType.subtract)
                    nc.vector.tensor_tensor(
                        d_h0o, A[64:96], Bt[64:96], mybir.AluOpType.add)
                    nc.vector.tensor_tensor(
                        d_h1o, A[96:128], Bt[96:128], mybir.AluOpType.add)

                # v for this row chunk: 4 sub r-tiles in one 2-bank psum
                psv = psp.tile([128, 4, J], F32, tag="S", name=f"pv_{rc}")
                for sub in range(RC // KT):
                    for c in range(n_ct):
                        nc.tensor.matmul(
                            psv[:, sub, :],
                            xall[rc][:, c, sub * KT:(sub + 1) * KT],
                            wv_sb[:, c, :],
                            start=(c == 0), stop=(c == n_ct - 1))
                for sub in range(RC // KT):
                    vt = v_sb[rc * (RC // KT) + sub]
                    nc.vector.tensor_copy(
                        vt.rearrange("p (l x) -> p l x", x=128)[:, :, 0:D],
                        psv[:, sub, :].rearrange("p (l d) -> p l d", l=HG))

            pavs = {}

            def emit_core(qc):
                """SDPA kt-loop for q-chunk qc, both head pairs concurrently."""
                nk = 4 * qc + 4
                qvs = [qALL[qc].rearrange("p (a b) t -> p a b t", b=2)[:, jt, :, :]
                       for jt in range(2)]
                pav = [psp.tile([128, 2, RC], F32, tag="A", name=f"av{jt}_{qc}")
                       for jt in range(2)]
                for kt in range(nk):
                    j = kt - 4 * qc
                    qoff = max(0, j) * KT
                    for jt in range(2):
                        ps_s = psp.tile([128, 2, RC], F32, tag="S",
                                        name=f"s{jt}_{qc}_{kt}")
                        kap = kALL[kt // 4][:, jt, (kt % 4) * KT:(kt % 4 + 1) * KT]
                        for lh in range(2):
                            nc.tensor.matmul(
                                ps_s[:, lh, qoff:RC], kap,
                                qvs[jt][:, lh, qoff:RC],
                                start=True, stop=(j < 0))
                            if j >= 0:
                                # diagonal tile: accumulate the causal
                                # penalty into the triangle block (cheap
                                # N=128 matmul; keeps masking on PE)
                                nc.tensor.matmul(
                                    ps_s[:, lh, qoff:qoff + KT],
                                    ident_sb[:, :], mpen_sb[:, :],
                                    start=False, stop=True,
                                    skip_group_check=True)
                        e = expp.tile([128, 2, RC], FP16, tag="e",
                                      name=f"e{jt}_{qc}_{kt}")
                        nc.scalar.activation(
                            e[:, :, qoff:RC], ps_s[:, :, qoff:RC],
                            mybir.ActivationFunctionType.Exp, scale=0.125)
                        for lh in range(2):
                            hcol = (2 * jt + lh) * 128
                            nc.tensor.matmul(
                                pav[jt][:, lh, qoff:RC],
                                v_sb[kt][:, hcol:hcol + 128],
                                e[:, lh, qoff:RC],
                                start=(kt == 0), stop=(kt == nk - 1))
                pavs[qc] = pav

            def emit_norm(qc):
                """normalize both head pairs. Stage pav's y rows and
                denominator rows to base-0 SBUF tiles (jt0 via ACT, jt1 via
                DVE, in parallel) so the psum accumulators release after the
                copies. reciprocal_approx_fast REQUIRES a base-partition-0
                fp32 SBUF input (partition-offset APs silently misread on
                HW). The scale mults run on Pool (SBUF-only), off DVE."""
                pav = pavs.pop(qc)
                # jt0 staging on ACT, jt1 on DVE; DVE FIFO ordered so recip0
                # runs as soon as ACT's den0 lands
                den0 = npool.tile([64, 2, RC], F32, tag="den0", name=f"dn0_{qc}")
                yu0 = npool.tile([64, 2, RC], FP16, tag="yu0", name=f"yu0_{qc}")
                den1 = npool.tile([64, 2, RC], F32, tag="den1", name=f"dn1_{qc}")
                yu1 = npool.tile([64, 2, RC], FP16, tag="yu1", name=f"yu1_{qc}")
                rec0 = npool.tile([64, 2, RC], F32, tag="rec0", name=f"r0_{qc}")
                rec1 = npool.tile([64, 2, RC], F32, tag="rec1", name=f"r1_{qc}")
                nc.scalar.copy(den0, pav[0][64:128, :, :])
                nc.scalar.copy(yu0, pav[0][0:64, :, :])
                nc.vector.tensor_copy(den1, pav[1][64:128, :, :])
                nc.vector.reciprocal_approx_fast(out=rec0, in_=den0)
                nc.vector.tensor_copy(yu1, pav[1][0:64, :, :])
                nc.vector.reciprocal_approx_fast(out=rec1, in_=den1)
                if debug and qc == 0:
                    nc.sync.dma_start(drec[:, :], rec0[:, :, :])
                for jt, (yu, rec) in enumerate(((yu0, rec0), (yu1, rec1))):
                    nc.gpsimd.tensor_tensor(
                        yT[jt][qc][0:64, :], yu[:, 0, :],
                        rec[:, 0, :], mybir.AluOpType.mult)
                    nc.gpsimd.tensor_tensor(
                        yT[jt][qc][64:128, :], yu[:, 1, :],
                        rec[:, 1, :], mybir.AluOpType.mult)

            def emit_proj(qc, ring="S", cast_eng="v"):
                """output projection partial for q-chunk qc + store.

                ring="A" (valid only when the pav accumulators are already
                released, i.e. the last q-chunk) moves the po psum off the
                S-ring; cast_eng picks DVE ("v"), ACT ("s"), or alternating
                ("vs") for the PSUM->fp16 cast."""
                for i, rt in enumerate(range(4 * qc, 4 * qc + 4)):
                    rs = slice(rt * 128, (rt + 1) * 128)
                    ro = (rt % 4) * 128
                    po = psp.tile([128, 2 * RC], F32, tag=ring, name=f"po_{rt}")
                    for nt in range(2):
                        ns = slice(nt * 512, (nt + 1) * 512)
                        nc.tensor.matmul(po[:, ns], yT[0][qc][:, ro:ro + 128],
                                         wp_sb[:, 0, ns], start=True, stop=False)
                        nc.tensor.matmul(po[:, ns], yT[1][qc][:, ro:ro + 128],
                                         wp_sb[:, 1, ns], start=False, stop=True)
                    o_sb = npool.tile([128, 2 * RC], FP16, tag="o_sb")
                    eng = cast_eng if len(cast_eng) == 1 else cast_eng[i % 2]
                    if eng == "v":
                        nc.vector.tensor_copy(o_sb, po)
                    else:
                        nc.scalar.copy(o_sb, po)
                    nc.gpsimd.dma_start(out[rs, :], o_sb)

            # ---------------- interleaved schedule ----------------
            # norm(qc) is emitted RIGHT AFTER core(qc): its DVE ops then sit
            # ahead of the next phase1's bulky RoPE work in the DVE FIFO, so
            # the pav accumulators release quickly for core(qc+1). PE covers
            # the norm chain with phase1/proj matmuls. proj(2) goes before
            # norm(3) so the PE stays warm while the last norm chain runs.
            emit_phase1(0)
            emit_phase1(1)
            emit_core(0)
            emit_norm(0)
            emit_phase1(2)
            emit_core(1)
            emit_norm(1)
            emit_phase1(3)
            emit_proj(0)
            emit_core(2)
            emit_norm(2)
            emit_proj(1)
            emit_core(3)
            emit_norm(3)
            # tail: proj2 matmuls cover norm3's chain on PE; proj2 casts on
            # ACT (free after the last exp) so DVE finishes norm3 unimpeded;
            # proj3 po tiles use the released pav banks (ring A) so the two
            # proj groups never contend for psum slots
            emit_proj(2, ring="S", cast_eng="s")
            emit_proj(3, ring="A", cast_eng="vs")

            if debug:
                nc.sync.dma_start(dq[:, :], qALL[0].rearrange("p a t -> p (a t)"))
                nc.sync.dma_start(dk[:, :], kALL[0].rearrange("p a t -> p (a t)"))
                nc.sync.dma_start(dv[:, :], v_sb[0])
                nc.sync.dma_start(dy0[:, :], yT[0][0])
                nc.sync.dma_start(dy3[:, :], yT[0][3])

    nc.finalize()
    return nc


def _host_inputs(x, Wqkv, Wproj):
    x = np.asarray(x, dtype=np.float32)
    Wqkv = np.asarray(Wqkv, dtype=np.float32)
    Wproj = np.asarray(Wproj, dtype=np.float32)

    # RoPE tables (match reference: theta_i = base^(-2i/D), freqs = outer(t, theta))
    dim_idx = np.arange(D // 2, dtype=np.float32)
    theta = 1.0 / (ROPE_BASE ** (2.0 * dim_idx / D))
    t = np.arange(T, dtype=np.float32)
    freqs = np.outer(t, theta).astype(np.float32)         # [T, 32]
    cos32 = np.cos(freqs).T.astype(np.float32)            # [32, T]
    sin32 = np.sin(freqs).T.astype(np.float32)
    t1_h = np.ascontiguousarray(np.tile(cos32, (4, 1)).astype(np.float16))
    t2_h = np.ascontiguousarray(np.tile(sin32, (4, 1)).astype(np.float16))

    # causal penalty for the diagonal 128x128 block: -200 where k > q makes
    # exp((s-200)/8) underflow fp16 to zero; ident is the stationary operand
    kk = np.arange(KT)[:, None]
    qq = np.arange(KT)[None, :]
    mpen_h = np.ascontiguousarray((kk > qq).astype(np.float16) * np.float16(-200.0))
    ident_h = np.ascontiguousarray(np.eye(KT, dtype=np.float16))

    # q/k column permutation: j-tile jt holds heads (2jt, 2jt+1) as
    # [h_e(32) h'_e(32) | h_o(32) h'_o(32)] (evens top half, odds bottom)
    def qk_perm(g):
        idx = np.empty(J, dtype=np.int64)
        for jt in range(2):
            for p in range(128):
                if p < 32:
                    lh, dd = 2 * jt, 2 * p
                elif p < 64:
                    lh, dd = 2 * jt + 1, 2 * (p - 32)
                elif p < 96:
                    lh, dd = 2 * jt, 2 * (p - 64) + 1
                else:
                    lh, dd = 2 * jt + 1, 2 * (p - 96) + 1
                idx[jt * 128 + p] = (4 * g + lh) * D + dd
        return idx

    xT = [np.ascontiguousarray(x[b].T.astype(np.float16)) for b in range(B)]
    in_maps = []
    for core in range(NCORES):
        g, b = core // 2, core % 2
        perm = qk_perm(g)
        wq_g = np.ascontiguousarray(Wqkv[:, perm].astype(np.float16))
        wk_g = np.ascontiguousarray(Wqkv[:, C + perm].astype(np.float16))
        vcols = np.arange(4 * g * D, 4 * g * D + J)
        wv_g = np.ascontiguousarray(Wqkv[:, 2 * C + vcols].astype(np.float16))
        wp_g = np.ascontiguousarray(
            Wproj[4 * g * D: 4 * g * D + J, :].astype(np.float16))
        in_maps.append({
            "xt": xT[b], "wq": wq_g, "wk": wk_g, "wv": wv_g, "wp": wp_g,
            "t1": t1_h, "t2": t2_h, "ident": ident_h, "mpen": mpen_h,
        })
    return in_maps


def kernel(x, Wqkv, bqkv, Wproj, bproj, _want_results=False):
    global _nc_cache
    if _nc_cache is None:
        _nc_cache = _build()
    in_maps = _host_inputs(x, Wqkv, Wproj)
    res = run_bass_kernel_spmd(_nc_cache, in_maps, list(range(NCORES)))

    bqkv = np.asarray(bqkv, dtype=np.float32)
    bproj = np.asarray(bproj, dtype=np.float32)
    out = np.zeros((B, T, C), dtype=np.float32)
    for core in range(NCORES):
        g, b = core // 2, core % 2
        out[b] += res.results[core]["out"]
    out += bproj[None, None, :]
    if _want_results:
        return out, res
    return out


# revision 31
# speedup vs baseline: 1.1552x; 1.1064x over previous
"""Causal self-attention with RoPE on 8 trn2 NeuronCores.

Sharding: core = (head_group g in 0..3) x (batch b in 0..1).
Each core computes qkv/RoPE/SDPA/proj for 4 heads of one batch and returns a
[T, C] partial of that batch's output (proj contracts only its 256 rows of
Wproj); the host sums the 4 head-group partials per batch and adds bproj.

Device dataflow (tuned for PE row economy + per-matmul latency):
  - all matmul inputs fp16 (x, Wqkv cast host-side); PSUM accumulates fp32
  - host passes xT = x[b].T; q^T/k^T come out as [d, t] tiles
  - Wq/Wk columns permuted head-contiguous per 128-row j-tile:
    [h_e(32) h_o(32) | h'_e(32) h'_o(32)], so RoPE is 2 full-tile mults
    (cos/sin tables replicated per 32-row block) + 4 strided adds that land
    the rotated values DIRECTLY in the score-ready layout - no fixup copies
  - ONE shared fp16 cos/sin table pair for q and k; the 1/sqrt(D) score scale
    is folded into the exp activation's free scale parameter (x*0.125)
  - qT is stored BLOCK-DIAGONAL [128, 4 slots, t]: slot 2jt holds head 2jt in
    rows 0:64 (rows 64:128 zero), slot 2jt+1 holds head 2jt+1 in rows 64:128.
    Scores for a head pair are then ONE [128]-contraction matmul with
    free=2*512 at full PE rate (vs 2 half-rate K=64 matmuls)
  - causal: diagonal k-tile j restricts score/exp/av APs to q >= j*128
    (0.53x dense, the per-q-tile ideal) and only the [128,128] triangle
    block gets a mask multiply (DVE, fp16 2x)
  - V stored [k, 4*(64 data | 64 ones)]: attn@V_aug gives y and the softmax
    denominator in one accumulating matmul; normalization is a DVE
    reciprocal (no ACT table swaps - ACT does exp only, all run long)
  - engine load balance: PE matmuls; ACT exp only; DVE RoPE/tri-mask/
    normalize; Pool(gpsimd) v-copy, psum->fp16 out casts, memsets; Sync
    carries in/out DMA triggers with inputs split across idle engine queues
    at startup (spreads descriptor-gen serialization)
  - PSUM: pav accumulators own tag A (2 slots); ALL transient psum tiles
    (phase1 q/k, psv, scores, proj) share the tag-S ring so no transient
    alloc ever blocks on a live accumulator
  - emission is software-pipelined (phase1/SDPA-core/normalize/proj
    interleaved) so PE work covers the exp chain and RoPE tails

No numerics tricks beyond fp16 inputs: exp without max-subtraction (scores
~N(0,1) after scale, far from fp32 overflow).
"""

import os
import sys

import numpy as np

for _p in ("/opt/trn_rl_repo", "/root/.axon_site/_ro/trn_rl_repo"):
    if os.path.isdir(_p) and _p not in sys.path:
        sys.path.append(_p)

import concourse.bass as bass  # noqa: E402
import concourse.mybir as mybir  # noqa: E402
import concourse.tile as tile  # noqa: E402
from concourse import bacc  # noqa: E402
from concourse.bass_utils import run_bass_kernel_spmd  # noqa: E402

B = 2
T = 2048
C = 1024
H = 16
D = 64
ROPE_BASE = 10000.0

HG = 4            # heads per core
J = HG * D        # 256 local qkv columns per tensor
NCORES = 8
RC = 512          # row chunk (phase 1 free dim / q chunk)
KT = 128          # k tile
F32 = mybir.dt.float32
FP16 = mybir.dt.float16

_nc_cache = None


def _bcast2(ap_2d, n):
    """[128, F] slice -> [128, n(bcast), F] via a zero-stride middle dim."""
    return bass.AP(
        tensor=ap_2d.tensor, offset=ap_2d.offset,
        ap=[ap_2d.ap[0], [0, n], ap_2d.ap[-1]])


def _build(debug=False):
    nc = bacc.Bacc(None, target_bir_lowering=False)

    xt = nc.dram_tensor("xt", [C, T], FP16, kind="ExternalInput")
    wq = nc.dram_tensor("wq", [C, J], FP16, kind="ExternalInput")
    wk = nc.dram_tensor("wk", [C, J], FP16, kind="ExternalInput")
    wv = nc.dram_tensor("wv", [C, J], FP16, kind="ExternalInput")
    wp = nc.dram_tensor("wp", [J, C], FP16, kind="ExternalInput")
    # trig tables: cos/sin rows replicated per 32-block, shared by q and k
    t1 = nc.dram_tensor("t1", [128, T], FP16, kind="ExternalInput")
    t2 = nc.dram_tensor("t2", [128, T], FP16, kind="ExternalInput")
    # causal penalty for the diagonal 128x128 score block, applied as an
    # extra accumulating matmul ident.T @ mpen (mpen = -200 where k > q):
    # exp then underflows masked entries to zero - no post-exp mask op at all
    ident = nc.dram_tensor("ident", [128, KT], FP16, kind="ExternalInput")
    mpen = nc.dram_tensor("mpen", [128, KT], FP16, kind="ExternalInput")
    out = nc.dram_tensor("out", [T, C], FP16, kind="ExternalOutput")
    if debug:
        dq = nc.dram_tensor("dq", [128, 4 * RC], FP16, kind="ExternalOutput")
        dk = nc.dram_tensor("dk", [128, 2 * RC], FP16, kind="ExternalOutput")
        dv = nc.dram_tensor("dv", [128, HG * 128], FP16, kind="ExternalOutput")
        dy0 = nc.dram_tensor("dy0", [128, RC], FP16, kind="ExternalOutput")
        dy3 = nc.dram_tensor("dy3", [128, RC], FP16, kind="ExternalOutput")
        drec = nc.dram_tensor("drec", [64, 2 * RC], F32, kind="ExternalOutput")

    n_rc = T // RC            # 4
    n_ct = C // 128           # 8 contraction tiles
    n_vt = T // KT            # 16 v tiles

    with tile.TileContext(nc) as tc:
        with (
            tc.tile_pool(name="persist", bufs=1) as persist,
            tc.tile_pool(name="xc", bufs=3) as xcp,
            tc.tile_pool(name="tmp", bufs=4) as tmpp,
            tc.tile_pool(name="expp", bufs=10) as expp,
            tc.tile_pool(name="npool", bufs=3) as npool,
            tc.tile_pool(name="ps", bufs=2, space="PSUM") as psp,
        ):
            # ---- persistent tiles ----
            wq_sb = persist.tile([128, n_ct, J], FP16, tag="wq")
            wk_sb = persist.tile([128, n_ct, J], FP16, tag="wk")
            wv_sb = persist.tile([128, n_ct, J], FP16, tag="wv")
            t1_sb = persist.tile([128, T], FP16, tag="t1")
            t2_sb = persist.tile([128, T], FP16, tag="t2")
            ident_sb = persist.tile([128, KT], FP16, tag="ident")
            mpen_sb = persist.tile([128, KT], FP16, tag="mpen")
            wp_sb = persist.tile([128, 2, C], FP16, tag="wp")

            # qALL[rc]: [128, 4 slots, RC] block-diagonal (see module doc)
            qALL = [persist.tile([128, 4, RC], FP16, tag=f"qA{r}", name=f"qA{r}")
                    for r in range(n_rc)]
            kALL = [persist.tile([128, 2, RC], FP16, tag=f"kA{r}", name=f"kA{r}")
                    for r in range(n_rc)]
            yT = [[persist.tile([128, RC], FP16, tag=f"yT{j}_{r}", name=f"yT{j}_{r}")
                   for r in range(n_rc)] for j in range(2)]
            # v tiles: [128, HG*128] fp16; head l data at cols l*128..+64, ones after
            v_sb = [persist.tile([128, HG * 128], FP16, tag=f"v{i}", name=f"v{i}")
                    for i in range(n_vt)]
            xall = [xcp.tile([128, n_ct, RC], FP16, tag="xc", name=f"x{r}")
                    for r in range(n_rc)]

            xtr = xt.rearrange("(co p) t -> p co t", p=128)

            # ---- input DMAs spread across the 3 DMA-capable engine queues
            # (sync/SP, scalar/ACT, gpsimd/Pool), strictly need-ordered AND
            # byte-balanced: each queue sustains only ~150-220GB/s, so the
            # ~7MB input must be split evenly; xall[0] is split by c-tile so
            # the first q accumulation starts as tiles arrive.
            nc.sync.dma_start(wq_sb, wq.rearrange("(co p) j -> p co j", p=128))
            nc.gpsimd.dma_start(xall[0][:, 0:4, :], xtr[:, 0:4, 0:RC])
            nc.sync.dma_start(xall[0][:, 4:8, :], xtr[:, 4:8, 0:RC])
            nc.scalar.dma_start(ident_sb, ident[:, :])
            nc.scalar.dma_start(mpen_sb, mpen[:, :])
            nc.scalar.dma_start(wk_sb, wk.rearrange("(co p) j -> p co j", p=128))
            nc.gpsimd.dma_start(t1_sb, t1[:, :])
            nc.scalar.dma_start(t2_sb, t2[:, :])
            nc.scalar.dma_start(wv_sb, wv.rearrange("(co p) j -> p co j", p=128))
            nc.gpsimd.dma_start(xall[1][:, 0:4, :], xtr[:, 0:4, RC:2 * RC])
            nc.sync.dma_start(xall[1][:, 4:8, :], xtr[:, 4:8, RC:2 * RC])
            nc.scalar.dma_start(wp_sb, wp.rearrange("(jt p) n -> p jt n", p=128))
            # xall[2], xall[3] prefetched inside the pipeline below (on sync)

            # zero the off-diagonal q half-blocks (disjoint from the RoPE
            # write region, so RoPE never waits on these) and set the V ones
            # columns (before phase 2). On Pool, ordered by first-use time.
            def ones_for(lo, hi):
                for i in range(lo, hi):
                    oap = v_sb[i].rearrange("p (l x) -> p l x", x=128)[:, :, D:128]
                    nc.gpsimd.memset(oap, 1.0)

            def qzero(r):
                qv = qALL[r].rearrange("p (a b) t -> p a b t", b=2)
                nc.gpsimd.memset(qv[64:128, :, 0, :], 0.0)
                nc.gpsimd.memset(qv[0:64, :, 1, :], 0.0)

            ones_for(0, 4)
            for r in range(n_rc):
                qzero(r)
            ones_for(4, n_vt)

            # ---------------- emission helpers ----------------
            def emit_phase1(rc):
                """qkv + RoPE for row chunk rc."""
                rcs = slice(rc * RC, (rc + 1) * RC)
                if rc + 2 < n_rc:
                    nc.sync.dma_start(
                        xall[rc + 2], xtr[:, :, (rc + 2) * RC:(rc + 3) * RC])

                for (w_sb, kind) in ((wq_sb, "q"), (wk_sb, "k")):
                    ps = psp.tile([128, 2, RC], F32, tag="S", name=f"p1{kind}_{rc}")
                    for jt in range(2):
                        for c in range(n_ct):
                            nc.tensor.matmul(
                                ps[:, jt, :],
                                w_sb[:, c, jt * 128:(jt + 1) * 128],
                                xall[rc][:, c, :],
                                start=(c == 0), stop=(c == n_ct - 1))
                    # RoPE on psum rows [h0e h1e | h0o h1o]:
                    #   A  = ps * cos           (natural rows)
                    #   B~ = swap64(ps) * sin   (2 half mults; PSUM-source
                    #                            partition shift is free)
                    # then 4 half adds with base-aligned SBUF inputs; the
                    # OUTPUT base is free, so results land head-contiguous
                    # ([he(32) ho(32)] per head) with no fixup copies.
                    T1s = _bcast2(t1_sb[:, rcs], 2)
                    T2s = _bcast2(t2_sb[:, rcs], 2)
                    T2lo = bass.AP(tensor=T2s.tensor, offset=T2s.offset,
                                   ap=[[T2s.ap[0][0], 64], [0, 2], [1, RC]])
                    A = tmpp.tile([128, 2, RC], FP16, tag="A", name=f"A{kind}{rc}")
                    Bt = tmpp.tile([128, 2, RC], FP16, tag="B", name=f"B{kind}{rc}")
                    nc.vector.tensor_tensor(A, ps, T1s, mybir.AluOpType.mult)
                    nc.vector.tensor_tensor(
                        Bt[0:64], ps[64:128], T2lo, mybir.AluOpType.mult)
                    nc.vector.tensor_tensor(
                        Bt[64:128], ps[0:64], T2lo, mybir.AluOpType.mult)
                    if kind == "q":
                        dsv = qALL[rc].rearrange("p (a b) t -> p a b t", b=2)
                        d_h0e = dsv[0:32, :, 0, :]
                        d_h0o = dsv[32:64, :, 0, :]
                        d_h1e = dsv[64:96, :, 1, :]
                        d_h1o = dsv[96:128, :, 1, :]
                    else:
                        d_h0e = kALL[rc][0:32, :, :]
                        d_h0o = kALL[rc][32:64, :, :]
                        d_h1e = kALL[rc][64:96, :, :]
                        d_h1o = kALL[rc][96:128, :, :]
                    # rows of A/B~: 0:32=h0e, 32:64=h1e, 64:96=h0o, 96:128=h1o
                    nc.vector.tensor_tensor(
                        d_h0e, A[0:32], Bt[0:32], mybir.AluOpType.subtract)
                    nc.vector.tensor_tensor(
                        d_h1e, A[32:64], Bt[32:64], mybir.AluOpType.subtract)
                    nc.vector.tensor_tensor(
                        d_h0o, A[64:96], Bt[64:96], mybir.AluOpType.add)
                    nc.vector.tensor_tensor(
                        d_h1o, A[96:128], Bt[96:128], mybir.AluOpType.add)

                # v for this row chunk: 4 sub r-tiles in one 2-bank psum.
                # Tag "A": with norm emitted right after each core, the pav
                # slot this lands in has just been released by the norm
                # copies - while tag "S" would stall the v matmuls on this
                # rc's OWN RoPE-q reads. v copies ride ACT (idle here),
                # keeping the DVE FIFO short for RoPE.
                psv = psp.tile([128, 4, J], F32, tag="A", name=f"pv_{rc}")
                for sub in range(RC // KT):
                    for c in range(n_ct):
                        nc.tensor.matmul(
                            psv[:, sub, :],
                            xall[rc][:, c, sub * KT:(sub + 1) * KT],
                            wv_sb[:, c, :],
                            start=(c == 0), stop=(c == n_ct - 1))
                for sub in range(RC // KT):
                    vt = v_sb[rc * (RC // KT) + sub]
                    nc.scalar.copy(
                        vt.rearrange("p (l x) -> p l x", x=128)[:, :, 0:D],
                        psv[:, sub, :].rearrange("p (l d) -> p l d", l=HG))

            pavs = {}

            def emit_core(qc):
                """SDPA kt-loop for q-chunk qc, both head pairs concurrently."""
                nk = 4 * qc + 4
                qvs = [qALL[qc].rearrange("p (a b) t -> p a b t", b=2)[:, jt, :, :]
                       for jt in range(2)]
                pav = [psp.tile([128, 2, RC], F32, tag="A", name=f"av{jt}_{qc}")
                       for jt in range(2)]
                for kt in range(nk):
                    j = kt - 4 * qc
                    qoff = max(0, j) * KT
                    for jt in range(2):
                        ps_s = psp.tile([128, 2, RC], F32, tag="S",
                                        name=f"s{jt}_{qc}_{kt}")
                        kap = kALL[kt // 4][:, jt, (kt % 4) * KT:(kt % 4 + 1) * KT]
                        for lh in range(2):
                            nc.tensor.matmul(
                                ps_s[:, lh, qoff:RC], kap,
                                qvs[jt][:, lh, qoff:RC],
                                start=True, stop=(j < 0))
                            if j >= 0:
                                # diagonal tile: accumulate the causal
                                # penalty into the triangle block (cheap
                                # N=128 matmul; keeps masking on PE)
                                nc.tensor.matmul(
                                    ps_s[:, lh, qoff:qoff + KT],
                                    ident_sb[:, :], mpen_sb[:, :],
                                    start=False, stop=True,
                                    skip_group_check=True)
                        e = expp.tile([128, 2, RC], FP16, tag="e",
                                      name=f"e{jt}_{qc}_{kt}")
                        nc.scalar.activation(
                            e[:, :, qoff:RC], ps_s[:, :, qoff:RC],
                            mybir.ActivationFunctionType.Exp, scale=0.125)
                        for lh in range(2):
                            hcol = (2 * jt + lh) * 128
                            nc.tensor.matmul(
                                pav[jt][:, lh, qoff:RC],
                                v_sb[kt][:, hcol:hcol + 128],
                                e[:, lh, qoff:RC],
                                start=(kt == 0), stop=(kt == nk - 1))
                pavs[qc] = pav

            def emit_norm(qc):
                """normalize both head pairs. Stage pav's y rows and
                denominator rows to base-0 SBUF tiles (jt0 via ACT, jt1 via
                DVE, in parallel) so the psum accumulators release after the
                copies. reciprocal_approx_fast REQUIRES a base-partition-0
                fp32 SBUF input (partition-offset APs silently misread on
                HW). The scale mults run on Pool (SBUF-only), off DVE."""
                pav = pavs.pop(qc)
                # jt0 staging on ACT, jt1 on DVE; DVE FIFO ordered so recip0
                # runs as soon as ACT's den0 lands
                den0 = npool.tile([64, 2, RC], F32, tag="den0", name=f"dn0_{qc}")
                yu0 = npool.tile([64, 2, RC], FP16, tag="yu0", name=f"yu0_{qc}")
                den1 = npool.tile([64, 2, RC], F32, tag="den1", name=f"dn1_{qc}")
                yu1 = npool.tile([64, 2, RC], FP16, tag="yu1", name=f"yu1_{qc}")
                rec0 = npool.tile([64, 2, RC], F32, tag="rec0", name=f"r0_{qc}")
                rec1 = npool.tile([64, 2, RC], F32, tag="rec1", name=f"r1_{qc}")
                # release pav as fast as possible (copies first on both
                # engines), then the reciprocals
                nc.scalar.copy(den0, pav[0][64:128, :, :])
                nc.scalar.copy(yu0, pav[0][0:64, :, :])
                nc.vector.tensor_copy(den1, pav[1][64:128, :, :])
                nc.vector.tensor_copy(yu1, pav[1][0:64, :, :])
                nc.vector.reciprocal_approx_fast(out=rec0, in_=den0)
                nc.vector.reciprocal_approx_fast(out=rec1, in_=den1)
                if debug and qc == 0:
                    nc.sync.dma_start(drec[:, :], rec0[:, :, :])
                for jt, (yu, rec) in enumerate(((yu0, rec0), (yu1, rec1))):
                    nc.gpsimd.tensor_tensor(
                        yT[jt][qc][0:64, :], yu[:, 0, :],
                        rec[:, 0, :], mybir.AluOpType.mult)
                    nc.gpsimd.tensor_tensor(
                        yT[jt][qc][64:128, :], yu[:, 1, :],
                        rec[:, 1, :], mybir.AluOpType.mult)

            def emit_proj(qc, ring="S", cast_eng="v"):
                """output projection partial for q-chunk qc + store.

                ring="A" (valid only when the pav accumulators are already
                released, i.e. the last q-chunk) moves the po psum off the
                S-ring; cast_eng picks DVE ("v"), ACT ("s"), or alternating
                ("vs") for the PSUM->fp16 cast."""
                for i, rt in enumerate(range(4 * qc, 4 * qc + 4)):
                    rs = slice(rt * 128, (rt + 1) * 128)
                    ro = (rt % 4) * 128
                    po = psp.tile([128, 2 * RC], F32, tag=ring, name=f"po_{rt}")
                    for nt in range(2):
                        ns = slice(nt * 512, (nt + 1) * 512)
                        nc.tensor.matmul(po[:, ns], yT[0][qc][:, ro:ro + 128],
                                         wp_sb[:, 0, ns], start=True, stop=False)
                        nc.tensor.matmul(po[:, ns], yT[1][qc][:, ro:ro + 128],
                                         wp_sb[:, 1, ns], start=False, stop=True)
                    o_sb = npool.tile([128, 2 * RC], FP16, tag="o_sb")
                    eng = cast_eng if len(cast_eng) == 1 else cast_eng[i % 2]
                    if eng == "v":
                        nc.vector.tensor_copy(o_sb, po)
                    else:
                        nc.scalar.copy(o_sb, po)
                    nc.gpsimd.dma_start(out[rs, :], o_sb)

            # ---------------- interleaved schedule ----------------
            # norm(qc) is emitted RIGHT AFTER core(qc): its DVE ops then sit
            # ahead of the next phase1's bulky RoPE work in the DVE FIFO, so
            # the pav accumulators release quickly for core(qc+1). PE covers
            # the norm chain with phase1/proj matmuls. proj(2) goes before
            # norm(3) so the PE stays warm while the last norm chain runs.
            emit_phase1(0)
            emit_phase1(1)
            emit_core(0)
            emit_norm(0)
            emit_phase1(2)
            emit_core(1)
            emit_norm(1)
            emit_phase1(3)
            emit_proj(0, cast_eng="v")
            emit_core(2)
            emit_norm(2)
            emit_proj(1, cast_eng="s")
            emit_core(3)
            emit_norm(3)
            # tail: proj2 matmuls cover norm3's chain on PE; proj2 casts on
            # ACT (free after the last exp) so DVE finishes norm3 unimpeded;
            # proj3 po tiles use the released pav banks (ring A) so the two
            # proj groups never contend for psum slots
            emit_proj(2, ring="S", cast_eng="s")
            emit_proj(3, ring="A", cast_eng="vs")

            if debug:
                nc.sync.dma_start(dq[:, :], qALL[0].rearrange("p a t -> p (a t)"))
                nc.sync.dma_start(dk[:, :], kALL[0].rearrange("p a t -> p (a t)"))
                nc.sync.dma_start(dv[:, :], v_sb[0])
                nc.sync.dma_start(dy0[:, :], yT[0][0])
                nc.sync.dma_start(dy3[:, :], yT[0][3])

    nc.finalize()
    return nc


def _host_inputs(x, Wqkv, Wproj):
    x = np.asarray(x, dtype=np.float32)
    Wqkv = np.asarray(Wqkv, dtype=np.float32)
    Wproj = np.asarray(Wproj, dtype=np.float32)

    # RoPE tables (match reference: theta_i = base^(-2i/D), freqs = outer(t, theta))
    dim_idx = np.arange(D // 2, dtype=np.float32)
    theta = 1.0 / (ROPE_BASE ** (2.0 * dim_idx / D))
    t = np.arange(T, dtype=np.float32)
    freqs = np.outer(t, theta).astype(np.float32)         # [T, 32]
    cos32 = np.cos(freqs).T.astype(np.float32)            # [32, T]
    sin32 = np.sin(freqs).T.astype(np.float32)
    t1_h = np.ascontiguousarray(np.tile(cos32, (4, 1)).astype(np.float16))
    t2_h = np.ascontiguousarray(np.tile(sin32, (4, 1)).astype(np.float16))

    # causal penalty for the diagonal 128x128 block: -200 where k > q makes
    # exp((s-200)/8) underflow fp16 to zero; ident is the stationary operand
    kk = np.arange(KT)[:, None]
    qq = np.arange(KT)[None, :]
    mpen_h = np.ascontiguousarray((kk > qq).astype(np.float16) * np.float16(-200.0))
    ident_h = np.ascontiguousarray(np.eye(KT, dtype=np.float16))

    # q/k column permutation: j-tile jt holds heads (2jt, 2jt+1) as
    # [h_e(32) h'_e(32) | h_o(32) h'_o(32)] (evens top half, odds bottom)
    def qk_perm(g):
        idx = np.empty(J, dtype=np.int64)
        for jt in range(2):
            for p in range(128):
                if p < 32:
                    lh, dd = 2 * jt, 2 * p
                elif p < 64:
                    lh, dd = 2 * jt + 1, 2 * (p - 32)
                elif p < 96:
                    lh, dd = 2 * jt, 2 * (p - 64) + 1
                else:
                    lh, dd = 2 * jt + 1, 2 * (p - 96) + 1
                idx[jt * 128 + p] = (4 * g + lh) * D + dd
        return idx

    xT = [np.ascontiguousarray(x[b].T.astype(np.float16)) for b in range(B)]
    in_maps = []
    for core in range(NCORES):
        g, b = core // 2, core % 2
        perm = qk_perm(g)
        wq_g = np.ascontiguousarray(Wqkv[:, perm].astype(np.float16))
        wk_g = np.ascontiguousarray(Wqkv[:, C + perm].astype(np.float16))
        vcols = np.arange(4 * g * D, 4 * g * D + J)
        wv_g = np.ascontiguousarray(Wqkv[:, 2 * C + vcols].astype(np.float16))
        wp_g = np.ascontiguousarray(
            Wproj[4 * g * D: 4 * g * D + J, :].astype(np.float16))
        in_maps.append({
            "xt": xT[b], "wq": wq_g, "wk": wk_g, "wv": wv_g, "wp": wp_g,
            "t1": t1_h, "t2": t2_h, "ident": ident_h, "mpen": mpen_h,
        })
    return in_maps


def kernel(x, Wqkv, bqkv, Wproj, bproj, _want_results=False):
    global _nc_cache
    if _nc_cache is None:
        _nc_cache = _build()
    in_maps = _host_inputs(x, Wqkv, Wproj)
    res = run_bass_kernel_spmd(_nc_cache, in_maps, list(range(NCORES)))

    bqkv = np.asarray(bqkv, dtype=np.float32)
    bproj = np.asarray(bproj, dtype=np.float32)
    out = np.zeros((B, T, C), dtype=np.float32)
    for core in range(NCORES):
        g, b = core // 2, core % 2
        out[b] += res.results[core]["out"]
    out += bproj[None, None, :]
    if _want_results:
        return out, res
    return out
